# revision 1
# baseline (speedup 1.0000x reference)
"""Trainium2 Bass kernel for nn_Blast: out = x @ (W0 + 1 bias^T) + bias
where W0 block (i_in, i_out) = Vt[i] @ diag(S[o,i]) @ U[o].

Factorized algorithm (per core, 256 tokens):
  midT[(o,r), tok] = sum_in A[in, (o,r)] * xT[in, tok]     (A = Vt*S, built on device)
  out[tok, oq]     = sum_r midT[(o,r), tok] * U''[o, r, q]

Layout: the 272 mid rows (16 o-blocks x 17) live at 32-aligned slots
(o -> psum group g=o//4, slot j=o%4, rows 32j..32j+16); A is zero-padded to
512 columns so the A-phase runs full-128-row matmuls (f32r forbids PE
subarray tiling, and only full-K matmul streams engage the PE's 2.4 GHz
activity monitor).

Bias trick: out = x@W0 + (rowsum(x)+1)*bias.  A has a 17th all-ones column
per o-block (-> rowsum in mid row 32j+16); each mid bank is opened by a
matmul writing 1.0 everywhere, so rank rows carry mid+1 and padding rows
carry 1.0; U'' row 16 = bias (multiplies rowsum+1), row 17 = -sum_r U[o,r]
(cancels the +1 pollution via the 1.0 padding row). U'' is zero-padded to
K=128 so the B-phase matmuls also run full-K (stay warm) and share one
weight load per group of four output blocks.

PE warmup: ~40 dummy full-K matmuls run during the input-DMA window; the
hardware activity monitor only unthrottles 1.2->2.4 GHz after ~a window of
contiguous full-K matmul activity, and low-K matmuls do not count.

Sharding: pure data-parallel over the 2048 tokens (8 cores x 256); the
small factors are replicated. x is fed pre-transposed (xT) from the host.
"""

import numpy as np

IN_DIM = 4096
OUT_DIM = 4096
BLOCK = 256
RANK = 16
B_IN = 16
B_OUT = 16
N_CORES = 8
TOK = 2048
TPC = TOK // N_CORES          # 256 tokens per core
RA = RANK + 1                 # 17: rank cols + rowsum col per o-block
KU = RANK + 2                 # 18: used rows of U'' per o-block
CP = 32                       # padded per-o column stride (32-aligned slots)
CAP = B_OUT * CP              # 512 padded columns of A
NCHUNK = IN_DIM // 128        # 32 K-chunks
NWARM = 28                    # PE warmup matmuls

_CACHE = {}

# test.py toggles; harness never touches these
TRACE = False
TRACE_DIR = None
LAST_RESULTS = None


def build_program():
    import concourse.mybir as mybir
    from concourse import bacc
    from concourse.tile import TileContext

    f32 = mybir.dt.float32
    f32r = mybir.dt.float32r

    nc = bacc.Bacc(trn_type="TRN2")
    xt_d = nc.dram_tensor("xt", (IN_DIM, TPC), f32r, kind="ExternalInput")
    vt_d = nc.dram_tensor("vt", (B_IN, BLOCK, CP), f32, kind="ExternalInput")
    s_d = nc.dram_tensor("s_flat", (1, B_IN * CAP), f32r, kind="ExternalInput")
    aship_d = nc.dram_tensor("aship", (B_IN // 2, 2 * 128, CAP), f32r, kind="ExternalInput")
    u_d = nc.dram_tensor("u_mat", (B_OUT, KU, BLOCK), f32r, kind="ExternalInput")
    w_d = nc.dram_tensor("wseed", (128, BLOCK), f32r, kind="ExternalInput")
    konst_d = nc.dram_tensor("konst", (1, 2 * TPC), f32r, kind="ExternalInput")
    out_d = nc.dram_tensor("out", (TPC, OUT_DIM), f32, kind="ExternalOutput")

    with TileContext(nc) as tc:
        from contextlib import ExitStack

        with ExitStack() as ctx:
            consts = ctx.enter_context(tc.tile_pool(name="consts", bufs=1))
            spool = ctx.enter_context(tc.tile_pool(name="spool", bufs=4))
            xpool = ctx.enter_context(tc.tile_pool(name="xpool", bufs=1))
            apool = ctx.enter_context(tc.tile_pool(name="apool", bufs=1))
            midsb = ctx.enter_context(tc.tile_pool(name="midsb", bufs=1))
            outsb = ctx.enter_context(tc.tile_pool(name="outsb", bufs=6))
            ps_mid = ctx.enter_context(
                tc.tile_pool(name="ps_mid", bufs=1, space="PSUM")
            )

            # ---- input loads ----
            # warm-up seed: first transfer on the sync queue
            wsb = consts.tile([128, BLOCK], f32r, name="wsb", tag="wsb")
            nc.sync.dma_start(out=wsb[:], in_=w_d[:])

            # memset can't produce f32r (ISA), so ones come via DMA:
            # konst = [ones(256) | zeros(256)]
            konst_sb = consts.tile([1, 2 * TPC], f32r, name="konst_sb", tag="konst_sb")
            nc.gpsimd.dma_start(out=konst_sb[:], in_=konst_d[:])
            ones_sb = konst_sb[0:1, 0:128]
            onestpc_sb = konst_sb[0:1, 0:TPC]

            s_sb = consts.tile([1, B_IN * CAP], f32r, name="s_sb", tag="s_sb")
            nc.gpsimd.dma_start(out=s_sb[:], in_=s_d[:])

            # all Vt chunks in one DMA: vt_all[p, i, h, r], h = 128-row half
            vt_all = consts.tile([128, B_IN * 2 * CP], f32, name="vt_all", tag="vt_all")
            nc.gpsimd.dma_start(
                out=vt_all[:].rearrange("p (i a r) -> p i a r", i=B_IN, a=2),
                in_=vt_d[:].rearrange("i (a p) r -> p i a r", p=128),
            )
            vt_v = vt_all[:].rearrange("p (i a r) -> p i a r", i=B_IN, a=2)

            # U'': usb[32*(o%4)+r, o*256+q] = U''[o,r,q]; one DMA per slot j
            usb = consts.tile([128, B_OUT * BLOCK], f32r, name="usb", tag="usb")
            for j in range(4):
                nc.gpsimd.dma_start(
                    out=usb[32 * j : 32 * j + KU, :]
                    .rearrange("r (g q) -> r g q", g=4)[:, :, j * BLOCK : (j + 1) * BLOCK],
                    in_=u_d[:].rearrange("(g jj) r q -> jj r g q", jj=4)[j],
                )

            # x^T chunk batches interleaved with shipped A chunks (even i)
            # on the sync queue; chunks for odd i are built on device below
            XGRP = 4
            xbatches = []
            ashipped = {}
            for b in range(NCHUNK // XGRP):
                xb = xpool.tile([128, XGRP * TPC], f32r, name=f"xb{b}", tag=f"xb{b}")
                nc.sync.dma_start(
                    out=xb[:].rearrange("p (k t) -> p k t", k=XGRP),
                    in_=xt_d[b * XGRP * 128 : (b + 1) * XGRP * 128, :].rearrange(
                        "(k p) t -> p k t", p=128
                    ),
                )
                xbatches.append(xb)
                i = 2 * b  # even i whose chunk pair ships whole
                if i < B_IN:
                    ab = apool.tile(
                        [128, 2 * CAP], f32r, name=f"ab{i}", tag=f"ab{i}"
                    )
                    # early pairs ride the sync ring between x batches; late
                    # pairs go via the GpSimd queue so the x tail isn't
                    # serialized behind them
                    eng = nc.sync if i <= 4 else nc.gpsimd
                    eng.dma_start(
                        out=ab[:].rearrange("p (two c) -> p two c", two=2),
                        in_=aship_d[i // 2].rearrange("(two p) c -> p two c", p=128),
                    )
                    ashipped[2 * i] = ab[:, 0:CAP]
                    ashipped[2 * i + 1] = ab[:, CAP : 2 * CAP]

            def xchunk(k):
                return xbatches[k // XGRP][:, (k % XGRP) * TPC : (k % XGRP + 1) * TPC]


            # ---- A-builds: S row broadcast (PE), stage (ACT), Vt*S (DVE/GPS)
            # These engines start as soon as s/vt land, overlapping the PE
            # warmup below; the A-phase then never waits on a build.
            midp = []
            abuilt = {}
            with tc.tile_pool(name="ps_pre", bufs=1, space="PSUM") as ps_pre:
                # ---- PE warmup while inputs stream in ----
                warm = ps_pre.tile([128, BLOCK], f32, name="warm", tag="warm", bufs=1)
                for w in range(NWARM):
                    nc.tensor.matmul(
                        warm[:],
                        lhsT=wsb[:, 0:128],
                        rhs=wsb[:],
                        start=True,
                        stop=True,
                        tile_position=(0, 0),
                    )

                for i in range(1, B_IN, 2):
                    sp = ps_pre.tile([128, CAP], f32, name="sp", tag="sp", bufs=3)
                    nc.tensor.matmul(
                        sp[:],
                        lhsT=ones_sb,
                        rhs=s_sb[0:1, i * CAP : (i + 1) * CAP],
                        start=True,
                        stop=True,
                        tile_position=(0, 0),
                    )
                    sps = spool.tile([128, CAP], f32, name="sps", tag="sps")
                    nc.scalar.copy(sps[:], sp[:])
                    for h in range(2):
                        k = 2 * i + h
                        a_t = apool.tile([128, CAP], f32r, name=f"a{k}", tag=f"a{k}")
                        eng = nc.gpsimd if i >= 13 else nc.vector
                        eng.tensor_mul(
                            a_t[:].rearrange("p (o r) -> p o r", r=CP),
                            vt_v[:, i, h, :]
                            .unsqueeze(1)
                            .broadcast_to([128, B_OUT, CP]),
                            sps[:].rearrange("p (o r) -> p o r", r=CP),
                        )
                        abuilt[k] = a_t

                # ---- open the mid banks with 1.0 everywhere ----
                for g in range(4):
                    mp = ps_mid.tile([128, TPC], f32, name=f"midp{g}", tag=f"midp{g}")
                    nc.tensor.matmul(
                        mp[:],
                        lhsT=ones_sb,
                        rhs=onestpc_sb,
                        start=True,
                        stop=False,
                        tile_position=(0, 0),
                    )
                    midp.append(mp)

                asbs = [
                    ashipped[k] if k in ashipped else abuilt[k]
                    for k in range(NCHUNK)
                ]

                # ---- phase A: midT accumulation over 32 K-chunks ----
                # a dummy warm matmul after every other chunk keeps the PE
                # activity monitor latched through DMA-starvation gaps
                for k in range(NCHUNK):
                    for g in range(4):
                        nc.tensor.matmul(
                            midp[g][:],
                            lhsT=asbs[k][:, g * 128 : (g + 1) * 128],
                            rhs=xchunk(k),
                            start=False,
                            stop=(k == NCHUNK - 1),
                            tile_position=(0, 0),
                        )
                    nfill = 2 if 6 <= k <= 24 else (1 if 2 <= k <= 26 else 0)
                    for _ in range(nfill):
                        nc.tensor.matmul(
                            warm[:],
                            lhsT=wsb[:, 0:128],
                            rhs=wsb[:],
                            start=True,
                            stop=True,
                            tile_position=(0, 0),
                        )

            # ---- midT to SBUF, one token-half at a time so phase B can
            # start on half 0 while half 1 still copies ----
            mids = []
            for g in range(4):
                ms = midsb.tile([128, TPC], f32r, name=f"mids{g}", tag=f"mids{g}")
                mids.append(ms)
            for tt in range(2):
                for g in range(4):
                    sl = (slice(None), slice(tt * 128, (tt + 1) * 128))
                    if (g + tt) % 2 == 0:
                        nc.scalar.copy(mids[g][sl], midp[g][sl])
                    else:
                        nc.vector.tensor_copy(mids[g][sl], midp[g][sl])

            # ---- phase B: out tiles [128 tok, 256 q], K=128 ----
            ps_out = ctx.enter_context(
                tc.tile_pool(name="ps_out", bufs=4, space="PSUM")
            )
            OGRP = 4  # o-blocks per output DMA; o//4 == g inside a group
            for tt in range(TPC // 128):
                for og in range(B_OUT // OGRP):
                    osb_t = outsb.tile(
                        [128, OGRP * BLOCK], f32, name="osb", tag="osb"
                    )
                    for oo in range(OGRP):
                        o = og * OGRP + oo
                        po = ps_out.tile([128, BLOCK], f32, name="po", tag="po")
                        j = o % 4
                        nc.tensor.matmul(
                            po[:],
                            lhsT=mids[o // 4][
                                32 * j : 32 * j + KU, tt * 128 : (tt + 1) * 128
                            ],
                            rhs=usb[
                                32 * j : 32 * j + KU, o * BLOCK : (o + 1) * BLOCK
                            ],
                            start=True,
                            stop=True,
                            tile_position=(32 * j, 0),
                        )
                        if o % 2 == 0:
                            nc.vector.tensor_copy(
                                osb_t[:, oo * BLOCK : (oo + 1) * BLOCK], po[:]
                            )
                        else:
                            nc.scalar.copy(
                                osb_t[:, oo * BLOCK : (oo + 1) * BLOCK], po[:]
                            )
                    nc.sync.dma_start(
                        out=out_d[
                            tt * 128 : (tt + 1) * 128,
                            og * OGRP * BLOCK : (og + 1) * OGRP * BLOCK,
                        ],
                        in_=osb_t[:],
                    )

    nc.compile()
    return nc


def prep_inputs(x, S, U, Vt, bias):
    """Host-side layout prep. Returns per-core input maps."""
    x = np.ascontiguousarray(np.asarray(x, dtype=np.float32))
    S = np.asarray(S, dtype=np.float32)
    U = np.asarray(U, dtype=np.float32)
    Vt = np.asarray(Vt, dtype=np.float32)
    bias = np.asarray(bias, dtype=np.float32)

    xt = np.ascontiguousarray(x.reshape(TOK, IN_DIM).T)  # (4096, 2048)

    vt_aug = np.zeros((B_IN, BLOCK, CP), np.float32)
    vt_aug[:, :, :RANK] = Vt
    vt_aug[:, :, RANK] = 1.0  # rowsum column

    # s_flat[0, i*CAP + o*CP + r] = S_aug[o, i, r]; pad r>=17 stays 0
    s_pad = np.zeros((B_IN, B_OUT, CP), np.float32)
    s_pad[:, :, :RANK] = S.transpose(1, 0, 2)
    s_pad[:, :, RANK] = 1.0  # rowsum column weight
    s_flat = np.ascontiguousarray(s_pad.reshape(1, B_IN * CAP))

    # row 16 multiplies mid row (rowsum+1) -> bias;  row 17 multiplies the
    # constant 1.0 padding row and cancels the +1 bank-init pollution of the
    # 16 rank rows: -sum_r U[o,r,:]
    bias_row = bias.reshape(B_OUT, 1, BLOCK)
    comp_row = -U.sum(axis=1, keepdims=True)  # (16, 1, 256)
    u_aug = np.ascontiguousarray(
        np.concatenate([U, bias_row, comp_row], axis=1)
    )  # (16, 18, 256)

    # shipped A chunk pairs (even i): A[(i,p),(o,r)] = vt_aug[i,p,r]*s_pad[i,o,r]
    a_even = np.einsum(
        "ipr,ior->ipor", vt_aug[0::2], s_pad[0::2]
    )  # (8, 256, 16, 32)
    aship = np.ascontiguousarray(a_even.reshape(B_IN // 2, 2 * 128, CAP))

    rng = np.random.default_rng(0)
    wseed = rng.standard_normal((128, BLOCK), dtype=np.float32)

    konst = np.zeros((1, 2 * TPC), np.float32)
    konst[0, :TPC] = 1.0

    in_maps = []
    for c in range(N_CORES):
        in_maps.append(
            {
                "xt": np.ascontiguousarray(xt[:, c * TPC : (c + 1) * TPC]),
                "vt": vt_aug,
                "s_flat": s_flat,
                "aship": aship,
                "u_mat": u_aug,
                "wseed": wseed,
                "konst": konst,
            }
        )
    return in_maps


def kernel(x, S, U, Vt, bias):
    global LAST_RESULTS
    from concourse.bass_utils import run_bass_kernel_spmd

    if "nc" not in _CACHE:
        _CACHE["nc"] = build_program()
    nc = _CACHE["nc"]

    in_maps = prep_inputs(x, S, U, Vt, bias)
    res = run_bass_kernel_spmd(
        nc, in_maps, list(range(N_CORES)), trace=TRACE, tmpdir=TRACE_DIR
    )
    LAST_RESULTS = res
    out = np.concatenate([res.results[c]["out"] for c in range(N_CORES)], axis=0)
    return out.reshape(2, TOK // 2, OUT_DIM)



# revision 2
# speedup vs baseline: 1.4732x; 1.4732x over previous
"""Trainium2 Bass kernel for nn_Blast: out = x @ (W0 + 1 bias^T) + bias
where W0 block (i_in, i_out) = Vt[i_in] @ diag(S[i_out, i_in]) @ U[i_out].

True 3-stage factorization in bf16 (16x fewer PE-streamed columns than the
merged Vt*S formulation):

  step1  y[(i,r), tok]  = Vt_blockdiag^T @ x        (32 MMs, K=128, N=256)
  step2  z[(o,r), tok]  = M2 @ y                    (8 MMs,  K=128, N=256)
  step3  out[tok, q]    = z_o^T @ U4_o              (16 MMs, K=128, N=512)

Layouts (per core, 256 tokens):
  y: 2 PSUM banks, row 16i+r; each step1 MM writes a 32-aligned pair-slot
     (i even -> weight cols 0-15, i odd -> 16-31, zero-padded) so M=32
     tile_position stays 32-granular.
  z: 4 PSUM banks, o -> bank o//4, rows 32(o%4)+r; row 32j+16 carries
     (rowsum(x)+1) seeded by a K=1 matmul (e-vec x rs1) that also opens the
     bank, so step3's U4 tables add (rowsum+1)*bias via their bias rows.
  U4: per bank g two [128,512] halves; half a maps o=4g+2a(+1) to disjoint
     512-col output segments with zero rows elsewhere, so one K=128 MM per
     (bank, tok-half, half) emits a full [128 tok, 512 q] output tile.

All HBM traffic is bf16 (2 MB in, 2 MB out per core); host pre-transposes x
into [partition, chunk, token] layout so every DMA is a contiguous 2D slice
with 2 KB per partition line. Sharding: data-parallel over the 2048 tokens.
"""

import numpy as np
import ml_dtypes

IN_DIM = 4096
OUT_DIM = 4096
BLOCK = 256
RANK = 16
B_IN = 16
B_OUT = 16
N_CORES = 8
TOK = 2048
TPC = TOK // N_CORES          # 256 tokens per core
NCHUNK = IN_DIM // 128        # 32 K-chunks
XGRP = 4                      # chunks per x DMA group
NWARM = 5                     # PE warmup matmuls (N=512)

BF16 = ml_dtypes.bfloat16

_CACHE = {}

# test.py toggles; harness never touches these
TRACE = False
TRACE_DIR = None
LAST_RESULTS = None


def build_program():
    import concourse.mybir as mybir
    from concourse import bacc
    from concourse.tile import TileContext

    f32 = mybir.dt.float32
    bf16 = mybir.dt.bfloat16

    nc = bacc.Bacc(trn_type="TRN2")
    xt_d = nc.dram_tensor("xt", (128, NCHUNK * TPC), bf16, kind="ExternalInput")
    w1_d = nc.dram_tensor("w1", (128, 1024), bf16, kind="ExternalInput")
    m2_d = nc.dram_tensor("m2", (128, 1024), bf16, kind="ExternalInput")
    u4_d = nc.dram_tensor("u4", (128, 4096), bf16, kind="ExternalInput")
    rsv_d = nc.dram_tensor("rsv", (1, 384), bf16, kind="ExternalInput")
    out_d = nc.dram_tensor("out", (TPC, OUT_DIM), bf16, kind="ExternalOutput")

    with TileContext(nc) as tc:
        from contextlib import ExitStack

        with ExitStack() as ctx:
            consts = ctx.enter_context(tc.tile_pool(name="consts", bufs=1))
            xpool = ctx.enter_context(tc.tile_pool(name="xpool", bufs=1))
            ypool = ctx.enter_context(tc.tile_pool(name="ypool", bufs=1))
            zpool = ctx.enter_context(tc.tile_pool(name="zpool", bufs=1))
            opool = ctx.enter_context(tc.tile_pool(name="opool", bufs=4))
            ps = ctx.enter_context(tc.tile_pool(name="ps", bufs=1, space="PSUM"))
            ps_out = ctx.enter_context(
                tc.tile_pool(name="ps_out", bufs=2, space="PSUM")
            )

            # ---- const loads (gpsimd queue) ----
            w1sb = consts.tile([128, 1024], bf16, name="w1sb", tag="w1sb")
            nc.gpsimd.dma_start(out=w1sb[:], in_=w1_d[:])
            rssb = consts.tile([1, 384], bf16, name="rssb", tag="rssb")
            nc.gpsimd.dma_start(out=rssb[:], in_=rsv_d[:])
            m2sb = consts.tile([128, 1024], bf16, name="m2sb", tag="m2sb")
            nc.gpsimd.dma_start(out=m2sb[:], in_=m2_d[:])
            u4sb = consts.tile([128, 4096], bf16, name="u4sb", tag="u4sb")
            nc.gpsimd.dma_start(out=u4sb[:], in_=u4_d[:])

            # ---- x stream (sync queue), 4 chunks per transfer ----
            xg = []
            for b in range(NCHUNK // XGRP):
                xb = xpool.tile([128, XGRP * TPC], bf16, name=f"xb{b}", tag=f"xb{b}")
                nc.sync.dma_start(
                    out=xb[:], in_=xt_d[:, b * XGRP * TPC : (b + 1) * XGRP * TPC]
                )
                xg.append(xb)

            # ---- PSUM tiles: 2 y banks + 4 z banks (+2 rotating out banks) ----
            yps = [
                ps.tile([128, 512], f32, name=f"yps{c}", tag=f"yps{c}")
                for c in range(2)
            ]
            zps = [
                ps.tile([128, 512], f32, name=f"zps{g}", tag=f"zps{g}")
                for g in range(4)
            ]

            # ---- PE warmup during DMA-in; targets the out banks ----
            for w in range(NWARM):
                warm = ps_out.tile([128, 512], f32, name="warm", tag="po")
                nc.tensor.matmul(
                    warm[:],
                    lhsT=w1sb[:, 0:128],
                    rhs=w1sb[:, 0:512],
                    start=True,
                    stop=True,
                    tile_position=(0, 0),
                )

            # ---- rs seed: open each z bank with rowsum+1 in rows 32j+16 ----
            for g in range(4):
                nc.tensor.matmul(
                    zps[g][:, 0:TPC],
                    lhsT=rssb[0:1, 256:384],
                    rhs=rssb[0:1, 0:TPC],
                    start=True,
                    stop=False,
                    tile_position=(0, 0),
                )

            ysb = [
                ypool.tile([128, TPC], bf16, name=f"ysb{c}", tag=f"ysb{c}")
                for c in range(2)
            ]
            zsb = [
                zpool.tile([128, TPC], bf16, name=f"zsb{g}", tag=f"zsb{g}")
                for g in range(4)
            ]

            def s2_chunk(c, stop):
                for g in range(4):
                    nc.tensor.matmul(
                        zps[g][:, 0:TPC],
                        lhsT=m2sb[:, (4 * c + g) * 128 : (4 * c + g + 1) * 128],
                        rhs=ysb[c][:],
                        start=False,
                        stop=stop,
                        tile_position=(0, 0),
                    )

            # ---- step 1: 32 chunk MMs into 32-aligned y pair-slots ----
            for k in range(NCHUNK):
                p = k // 4
                bank, co = k // 16, 32 * (p % 4)
                nc.tensor.matmul(
                    yps[bank][co : co + 32, 0:TPC],
                    lhsT=w1sb[:, 32 * k : 32 * k + 32],
                    rhs=xg[k // 4][:, (k % 4) * TPC : (k % 4 + 1) * TPC],
                    start=(k % 4 == 0),
                    stop=(k % 4 == 3),
                    tile_position=(0, co),
                )
                if k == 15:
                    nc.scalar.copy(ysb[0][:], yps[0][:, 0:TPC])
                if k == 19:
                    s2_chunk(0, stop=False)  # hidden under the x stream
            nc.vector.tensor_copy(ysb[1][:], yps[1][:, 0:TPC])
            s2_chunk(1, stop=True)
            for g in range(4):
                eng = nc.scalar.copy if g % 2 == 0 else nc.vector.tensor_copy
                eng(zsb[g][:], zps[g][:, 0:TPC])

            # ---- step 3: [128 tok, 512 q] tiles; pair q-segments per DMA ----
            for tt in range(2):
                for s2i in range(4):
                    osb_t = opool.tile([128, 1024], bf16, name="osb", tag="osb")
                    for half in range(2):
                        s = 2 * s2i + half
                        g, a = s // 2, s % 2
                        po = ps_out.tile([128, 512], f32, name="po", tag="po")
                        nc.tensor.matmul(
                            po[:],
                            lhsT=zsb[g][:, tt * 128 : (tt + 1) * 128],
                            rhs=u4sb[:, g * 1024 + a * 512 : g * 1024 + (a + 1) * 512],
                            start=True,
                            stop=True,
                            tile_position=(0, 0),
                        )
                        eng = nc.vector.tensor_copy if s % 2 == 0 else nc.scalar.copy
                        eng(osb_t[:, half * 512 : (half + 1) * 512], po[:])
                    eng_dma = nc.gpsimd if tt == 0 else nc.sync
                    eng_dma.dma_start(
                        out=out_d[
                            tt * 128 : (tt + 1) * 128,
                            s2i * 1024 : (s2i + 1) * 1024,
                        ],
                        in_=osb_t[:],
                    )

    nc.compile()
    return nc


def prep_inputs(x, S, U, Vt, bias):
    """Host-side layout prep. Returns per-core input maps (all bf16)."""
    S = np.asarray(S, dtype=np.float32)
    U = np.asarray(U, dtype=np.float32)
    Vt = np.asarray(Vt, dtype=np.float32)
    bias = np.asarray(bias, dtype=np.float32)
    Xf = np.asarray(x, dtype=np.float32).reshape(TOK, IN_DIM)

    rowsum = Xf.sum(axis=1)
    xt_all = np.ascontiguousarray(Xf.T).astype(BF16)  # [4096, 2048]

    # step-1 weights: chunk k -> cols [32k, 32k+32), halves by i parity
    w1 = np.zeros((128, 1024), np.float32)
    for k in range(NCHUNK):
        i, h = k // 2, k % 2
        half = i % 2
        w1[:, 32 * k + 16 * half : 32 * k + 16 * half + 16] = Vt[
            i, 128 * h : 128 * h + 128, :
        ]

    # step-2 S-mixing blocks: (c,g) block maps y rows 16i'+r -> z rows 32j+r
    m2 = np.zeros((128, 1024), np.float32)
    r_idx = np.arange(RANK)
    for c in range(2):
        for g in range(4):
            blk = np.zeros((128, 128), np.float32)
            for ip in range(8):
                for j in range(4):
                    blk[16 * ip + r_idx, 32 * j + r_idx] = S[4 * g + j, 8 * c + ip, :]
            m2[:, (4 * c + g) * 128 : (4 * c + g + 1) * 128] = blk

    # step-3 tables: bank g, half a covers o = 4g+2a, 4g+2a+1
    u4 = np.zeros((128, 4096), np.float32)
    for g in range(4):
        for a in range(2):
            for b_ in range(2):
                j = 2 * a + b_
                o = 4 * g + j
                cols = slice(
                    g * 1024 + a * 512 + b_ * 256,
                    g * 1024 + a * 512 + b_ * 256 + 256,
                )
                u4[32 * j : 32 * j + RANK, cols] = U[o]
                u4[32 * j + RANK, cols] = bias[256 * o : 256 * o + 256]

    w1 = w1.astype(BF16)
    m2 = m2.astype(BF16)
    u4 = u4.astype(BF16)

    in_maps = []
    for c in range(N_CORES):
        xt_c = np.ascontiguousarray(
            xt_all[:, TPC * c : TPC * (c + 1)]
            .reshape(NCHUNK, 128, TPC)
            .transpose(1, 0, 2)
            .reshape(128, NCHUNK * TPC)
        )
        rsv = np.zeros((1, 384), np.float32)
        rsv[0, :TPC] = rowsum[TPC * c : TPC * (c + 1)] + 1.0
        rsv[0, 256 + np.array([16, 48, 80, 112])] = 1.0
        in_maps.append(
            {
                "xt": xt_c,
                "w1": w1,
                "m2": m2,
                "u4": u4,
                "rsv": rsv.astype(BF16),
            }
        )
    return in_maps


def kernel(x, S, U, Vt, bias):
    global LAST_RESULTS
    from concourse.bass_utils import run_bass_kernel_spmd

    if "nc" not in _CACHE:
        _CACHE["nc"] = build_program()
    nc = _CACHE["nc"]

    in_maps = prep_inputs(x, S, U, Vt, bias)
    res = run_bass_kernel_spmd(
        nc, in_maps, list(range(N_CORES)), trace=TRACE, tmpdir=TRACE_DIR
    )
    LAST_RESULTS = res
    out = np.concatenate(
        [np.asarray(res.results[c]["out"], dtype=np.float32) for c in range(N_CORES)],
        axis=0,
    )
    return out.reshape(2, TOK // 2, OUT_DIM)


# revision 4
# speedup vs baseline: 1.6309x; 1.1071x over previous
"""Trainium2 Bass kernel for nn_Blast: out = x @ (W0 + 1 bias^T) + bias
where W0 block (i_in, i_out) = Vt[i_in] @ diag(S[i_out, i_in]) @ U[i_out].

True 3-stage factorization in bf16 (16x fewer PE-streamed columns than the
merged Vt*S formulation):

  step1  y[(i,r), tok]  = Vt_blockdiag^T @ x        (32 MMs, K=128, N=256)
  step2  z[(o,r), tok]  = M2 @ y                    (8 MMs,  K=128, N=256)
  step3  out[tok, q]    = z_o^T @ U4_o              (16 MMs, K=128, N=512)

Layouts (per core, 256 tokens):
  y: 2 PSUM banks, row 16i+r; each step1 MM writes a 32-aligned pair-slot
     (i even -> weight cols 0-15, i odd -> 16-31, zero-padded) so M=32
     tile_position stays 32-granular.
  z: 2 PSUM banks, two [128,256] tok-tiles per bank (z01 | z23); o -> tile
     o//4... wait see code; rows 32(o%4)+r; row 32(o%4)+16 carries
     (rowsum(x)+1) seeded by K=1 matmuls (e-vec x rs1) that also open each
     bank, so step3's U4 tables add (rowsum+1)*bias via their bias rows.
  U4: per z-tile g two [128,512] halves; half a maps o=4g+2a(+1) to disjoint
     512-col output segments with zero rows elsewhere, so one K=128 MM per
     (g, tok-half, half) emits a full [128 tok, 512 q] output tile.
  out: 2 rotating [128,1024] PSUM pair-tiles (2 banks each); one engine copy
     per pair evacuates to SBUF bf16.

All HBM traffic is bf16 (2 MB in, 2 MB out per core); host pre-transposes x
into [partition, chunk, token] layout so every DMA is a contiguous 2D slice
with 2 KB per partition line. x streams on two queues (sync+gpsimd); consts
ride scalar/vector queues. NWARM matmuls run during the DMA window to lift
the PE clock gate (HAM) from 1.2 to 2.4 GHz before real work arrives.
Sharding: data-parallel over the 2048 tokens.
"""

import numpy as np
import ml_dtypes

IN_DIM = 4096
OUT_DIM = 4096
BLOCK = 256
RANK = 16
B_IN = 16
B_OUT = 16
N_CORES = 8
TOK = 2048
TPC = TOK // N_CORES          # 256 tokens per core
NCHUNK = IN_DIM // 128        # 32 K-chunks
XGRP = 4                      # chunks per x DMA group
NWARM = 8                     # PE warmup matmuls (N=512)
FILLS = {11: 1, 19: 1}        # chunk -> extra keep-warm MMs after it

BF16 = ml_dtypes.bfloat16

_CACHE = {}

# test.py toggles; harness never touches these
TRACE = False
TRACE_DIR = None
LAST_RESULTS = None


def build_program():
    import concourse.mybir as mybir
    from concourse import bacc
    from concourse.tile import TileContext

    f32 = mybir.dt.float32
    bf16 = mybir.dt.bfloat16

    nc = bacc.Bacc(trn_type="TRN2")
    xt_d = nc.dram_tensor("xt", (128, NCHUNK * TPC), bf16, kind="ExternalInput")
    w1_d = nc.dram_tensor("w1", (128, 1024), bf16, kind="ExternalInput")
    m2_d = nc.dram_tensor("m2", (128, 1024), bf16, kind="ExternalInput")
    u4_d = nc.dram_tensor("u4", (128, 4096), bf16, kind="ExternalInput")
    rsv_d = nc.dram_tensor("rsv", (1, 384), bf16, kind="ExternalInput")
    out_d = nc.dram_tensor("out", (TPC, OUT_DIM), bf16, kind="ExternalOutput")

    with TileContext(nc) as tc:
        from contextlib import ExitStack

        with ExitStack() as ctx:
            consts = ctx.enter_context(tc.tile_pool(name="consts", bufs=1))
            xpool = ctx.enter_context(tc.tile_pool(name="xpool", bufs=1))
            ypool = ctx.enter_context(tc.tile_pool(name="ypool", bufs=1))
            zpool = ctx.enter_context(tc.tile_pool(name="zpool", bufs=1))
            opool = ctx.enter_context(tc.tile_pool(name="opool", bufs=4))
            ps = ctx.enter_context(tc.tile_pool(name="ps", bufs=1, space="PSUM"))
            ps_out = ctx.enter_context(
                tc.tile_pool(name="ps_out", bufs=2, space="PSUM")
            )

            # ---- const loads: w1+m2+rsv on scalar queue, u4 on vector ----
            w1sb = consts.tile([128, 1024], bf16, name="w1sb", tag="w1sb")
            nc.scalar.dma_start(out=w1sb[:], in_=w1_d[:])
            rssb = consts.tile([1, 384], bf16, name="rssb", tag="rssb")
            nc.scalar.dma_start(out=rssb[:], in_=rsv_d[:])
            m2sb = consts.tile([128, 1024], bf16, name="m2sb", tag="m2sb")
            nc.scalar.dma_start(out=m2sb[:], in_=m2_d[:])
            u4sb = consts.tile([128, 4096], bf16, name="u4sb", tag="u4sb")
            nc.gpsimd.dma_start(out=u4sb[:], in_=u4_d[:])

            # ---- x stream: 4 chunks per transfer; groups 5,7 ride gpsimd
            # behind u4 so both queues finish together ----
            xg = []
            for b in range(NCHUNK // XGRP):
                xb = xpool.tile([128, XGRP * TPC], bf16, name=f"xb{b}", tag=f"xb{b}")
                eng = nc.gpsimd if b in (5, 7) else nc.sync
                eng.dma_start(
                    out=xb[:], in_=xt_d[:, b * XGRP * TPC : (b + 1) * XGRP * TPC]
                )
                xg.append(xb)

            # ---- PSUM: 2 y banks + 2 z banks (2 tok-tiles each) ----
            yps = [
                ps.tile([128, 512], f32, name=f"yps{c}", tag=f"yps{c}")
                for c in range(2)
            ]
            zps = [
                ps.tile([128, 512], f32, name=f"zpsb{b}", tag=f"zpsb{b}")
                for b in range(2)
            ]

            def zv(g):  # z tile for group g: bank g//2, half g%2
                return zps[g // 2][:, (g % 2) * TPC : (g % 2 + 1) * TPC]

            # ---- PE warmup during DMA-in; targets the out banks ----
            for w in range(NWARM):
                warm = ps_out.tile([128, 1024], f32, name="warm", tag="po")
                nc.tensor.matmul(
                    warm[:, 0:512],
                    lhsT=w1sb[:, 0:128],
                    rhs=w1sb[:, 0:512],
                    start=True,
                    stop=True,
                    tile_position=(0, 0),
                )

            # ---- rs seed: open z banks with rowsum+1 in rows 32j+16 ----
            # start=True only on the first MM touching each bank; the clear
            # covers the sibling half, whose first write then overwrites.
            for g in range(4):
                nc.tensor.matmul(
                    zv(g),
                    lhsT=rssb[0:1, 256:384],
                    rhs=rssb[0:1, 0:TPC],
                    start=(g % 2 == 0),
                    stop=False,
                    tile_position=(0, 0),
                )

            ysb = [
                ypool.tile([128, TPC], bf16, name=f"ysb{c}", tag=f"ysb{c}")
                for c in range(2)
            ]
            zsb = [
                zpool.tile([128, 2 * TPC], bf16, name=f"zsb{b}", tag=f"zsb{b}")
                for b in range(2)
            ]

            def s2_chunk(c, stop):
                for g in range(4):
                    nc.tensor.matmul(
                        zv(g),
                        lhsT=m2sb[:, (4 * c + g) * 128 : (4 * c + g + 1) * 128],
                        rhs=ysb[c][:],
                        start=False,
                        stop=stop,
                        tile_position=(0, 0),
                    )

            # ---- step 1: 32 chunk MMs into 32-aligned y pair-slots ----
            for k in range(NCHUNK):
                p = k // 4
                bank, co = k // 16, 32 * (p % 4)
                nc.tensor.matmul(
                    yps[bank][co : co + 32, 0:TPC],
                    lhsT=w1sb[:, 32 * k : 32 * k + 32],
                    rhs=xg[k // 4][:, (k % 4) * TPC : (k % 4 + 1) * TPC],
                    start=(k % 4 == 0),
                    stop=(k % 4 == 3),
                    tile_position=(0, co),
                )
                for _ in range(FILLS.get(k, 0)):
                    warm = ps_out.tile([128, 1024], f32, name="warm", tag="po")
                    nc.tensor.matmul(
                        warm[:, 0:512],
                        lhsT=w1sb[:, 0:128],
                        rhs=w1sb[:, 0:512],
                        start=True,
                        stop=True,
                        tile_position=(0, 0),
                    )
                if k == 15:
                    nc.scalar.copy(ysb[0][:], yps[0][:, 0:TPC])
                if k == 19:
                    s2_chunk(0, stop=False)  # hidden under the x stream
            nc.vector.tensor_copy(ysb[1][:], yps[1][:, 0:TPC])
            s2_chunk(1, stop=True)
            # one copy per z bank; PE is done with z banks by then
            nc.scalar.copy(zsb[0][:], zps[0][:])
            nc.vector.tensor_copy(zsb[1][:], zps[1][:])

            def zslice(g, tt):
                return zsb[g // 2][:, (g % 2) * TPC + tt * 128 : (g % 2) * TPC + (tt + 1) * 128]

            # ---- step 3: [128 tok, 2x512 q] pair tiles; one copy per pair ----
            for tt in range(2):
                for s2i in range(4):
                    osb_t = opool.tile([128, 1024], bf16, name="osb", tag="osb")
                    po = ps_out.tile([128, 1024], f32, name="po", tag="po")
                    for half in range(2):
                        s = 2 * s2i + half
                        g, a = s // 2, s % 2
                        nc.tensor.matmul(
                            po[:, half * 512 : (half + 1) * 512],
                            lhsT=zslice(g, tt),
                            rhs=u4sb[:, g * 1024 + a * 512 : g * 1024 + (a + 1) * 512],
                            start=True,
                            stop=True,
                            tile_position=(0, 0),
                        )
                    eng = nc.vector.tensor_copy if s2i % 2 == 0 else nc.scalar.copy
                    eng(osb_t[:], po[:])
                    eng_dma = nc.gpsimd if tt == 0 else nc.sync
                    eng_dma.dma_start(
                        out=out_d[
                            tt * 128 : (tt + 1) * 128,
                            s2i * 1024 : (s2i + 1) * 1024,
                        ],
                        in_=osb_t[:],
                    )

    nc.compile()
    return nc


def prep_inputs(x, S, U, Vt, bias):
    """Host-side layout prep. Returns per-core input maps (all bf16)."""
    S = np.asarray(S, dtype=np.float32)
    U = np.asarray(U, dtype=np.float32)
    Vt = np.asarray(Vt, dtype=np.float32)
    bias = np.asarray(bias, dtype=np.float32)
    Xf = np.asarray(x, dtype=np.float32).reshape(TOK, IN_DIM)

    rowsum = Xf.sum(axis=1)
    xt_all = np.ascontiguousarray(Xf.T).astype(BF16)  # [4096, 2048]

    # step-1 weights: chunk k -> cols [32k, 32k+32), halves by i parity
    w1 = np.zeros((128, 1024), np.float32)
    for k in range(NCHUNK):
        i, h = k // 2, k % 2
        half = i % 2
        w1[:, 32 * k + 16 * half : 32 * k + 16 * half + 16] = Vt[
            i, 128 * h : 128 * h + 128, :
        ]

    # step-2 S-mixing blocks: (c,g) block maps y rows 16i'+r -> z rows 32j+r
    m2 = np.zeros((128, 1024), np.float32)
    r_idx = np.arange(RANK)
    for c in range(2):
        for g in range(4):
            blk = np.zeros((128, 128), np.float32)
            for ip in range(8):
                for j in range(4):
                    blk[16 * ip + r_idx, 32 * j + r_idx] = S[4 * g + j, 8 * c + ip, :]
            m2[:, (4 * c + g) * 128 : (4 * c + g + 1) * 128] = blk

    # step-3 tables: z-tile g, half a covers o = 4g+2a, 4g+2a+1
    u4 = np.zeros((128, 4096), np.float32)
    for g in range(4):
        for a in range(2):
            for b_ in range(2):
                j = 2 * a + b_
                o = 4 * g + j
                cols = slice(
                    g * 1024 + a * 512 + b_ * 256,
                    g * 1024 + a * 512 + b_ * 256 + 256,
                )
                u4[32 * j : 32 * j + RANK, cols] = U[o]
                u4[32 * j + RANK, cols] = bias[256 * o : 256 * o + 256]

    w1 = w1.astype(BF16)
    m2 = m2.astype(BF16)
    u4 = u4.astype(BF16)

    in_maps = []
    for c in range(N_CORES):
        xt_c = np.ascontiguousarray(
            xt_all[:, TPC * c : TPC * (c + 1)]
            .reshape(NCHUNK, 128, TPC)
            .transpose(1, 0, 2)
            .reshape(128, NCHUNK * TPC)
        )
        rsv = np.zeros((1, 384), np.float32)
        rsv[0, :TPC] = rowsum[TPC * c : TPC * (c + 1)] + 1.0
        rsv[0, 256 + np.array([16, 48, 80, 112])] = 1.0
        in_maps.append(
            {
                "xt": xt_c,
                "w1": w1,
                "m2": m2,
                "u4": u4,
                "rsv": rsv.astype(BF16),
            }
        )
    return in_maps


def kernel(x, S, U, Vt, bias):
    global LAST_RESULTS
    from concourse.bass_utils import run_bass_kernel_spmd

    if "nc" not in _CACHE:
        _CACHE["nc"] = build_program()
    nc = _CACHE["nc"]

    in_maps = prep_inputs(x, S, U, Vt, bias)
    res = run_bass_kernel_spmd(
        nc, in_maps, list(range(N_CORES)), trace=TRACE, tmpdir=TRACE_DIR
    )
    LAST_RESULTS = res
    out = np.concatenate(
        [np.asarray(res.results[c]["out"], dtype=np.float32) for c in range(N_CORES)],
        axis=0,
    )
    return out.reshape(2, TOK // 2, OUT_DIM)
